# revision 55
# baseline (speedup 1.0000x reference)
"""Trainium2 Bass kernel for DigitConvolutionalModel.

Math: the 3x3 valid conv on the 28x28 image is a linear map, so it folds into
the first Linear layer:
    out = relu(x @ W_eff + b1) @ w2.T + b2
where W_eff[784, 128] = C @ w1.T and C[784, 676] is the conv-as-matrix built
from conv_w.  W_eff is built on the host (O(1) w.r.t. batch); the device does
the two batch matmuls.

Distribution: pure data parallel — batch dim of x sharded across 8 NeuronCores,
weights replicated.  Each core computes out.T [10, 8192]; the host reassembles
[65536, 10].

dtypes: x ships as fp8 e3m4 (4 mantissa bits) — the tolerance is 2e-2 and
e3m4-x against fp16 weights measures 1.3e-2 end-to-end, so this halves HBM
traffic and makes the kernel PE-bound.  The PE takes mixed-dtype operands
(stationary W fp16, moving x fp8) at the full 1 col/cycle rate; accumulation
is fp32 in PSUM.

PE uniformity: every matmul is a full 128x128xN=512 pass.  The 16 remainder
features (768..784) ride a 7th zero-padded K-tile (zero weight rows x zero
x rows) instead of a K=16 matmul, and w2 is zero-padded to [128, 128] — a
K=16 or M=10 matmul switches the PE row/col group config, which costs ~100ns
on each matmul around the transition (measured: 313ns vs 216ns issue gap).

Schedule: all 16 batch tiles are SBUF-resident (56KB/partition at fp8) and
every DMA trigger is emitted up-front.  sync ring: tile 0 in two k-chunks,
solo tiles 2/4, pairs, then the output stores.  scalar ring: weights (one
contiguous DMA), solo tiles 1/3/5, then pairs.  Tiny biases ride SWDGE.
The PE's first matmul fires ~3.5us after the entry barrier; an N=128
warmup stream bridges the HAM clock-ramp window until tile 0 lands, so
real matmuls run at the warm 2.4GHz 216ns/512-col rate throughout.  The
epilogue (relu+bias on DVE, mm2, +b2 on ScalarE) is software-pipelined one
tile behind mm1 and split in halves for the last two tiles; the output
lands in four 32-aligned partition bands and is stored in four column
chunks so only a 128KB store trails the last matmul.
"""

import numpy as np
import ml_dtypes

import concourse.bass as bass  # noqa: F401  (bass registers mybir lowerings)
import concourse.mybir as mybir
import concourse.tile as tile
from concourse import bacc
from concourse.bass_utils import run_bass_kernel_spmd

N_CORES = 8
B = 65536
B_SH = B // N_CORES  # 8192 rows per core
D = 784              # 28*28 input features
H = 128              # hidden
OUT = 10
KT = 128             # contraction tile = full partition dim
NK = 7               # K-tiles: 6 full + 1 zero-padded (features 768..784)
NB = 512             # batch columns per tile (= one fp32 PSUM bank)
NT = B_SH // NB      # 16 batch tiles
WARMUP = 40          # HAM clock-ramp matmuls (N=128) before real data arrives
WARM_N = 128         # small free dim: fine-grained busy bridging, ~107ns each

X_DT = "f8"          # "f8" (e3m4) or "f16" for the x stream

_CACHE = {}


def _build_nc():
    f32 = mybir.dt.float32
    f16 = mybir.dt.float16
    fx = mybir.dt.float8e3 if X_DT == "f8" else f16
    nc = bacc.Bacc("TRN2", target_bir_lowering=False, debug=False,
                   num_devices=N_CORES)
    # main x, partition-major: [p, t, k, c] with feature f = k*128 + p;
    # k=6 carries the 16 remainder features on partitions 0..15, zeros above
    xtp = nc.dram_tensor("xtp", [KT, NT, NK, NB], fx,
                         kind="ExternalInput").ap()
    # weights pre-rearranged on host: wtp[p, k, :] = W_eff[k*128+p, :] for
    # k<7; slot 7 is w2 zero-padded to M=128 (full-array col-group for mm2).
    # One contiguous 2KB-per-partition DMA — separate small transfers would
    # be RMW-slow (<512B/partition) and block the ring FIFO.
    wtp = nc.dram_tensor("wtp", [KT, NK + 1, H], f16,
                         kind="ExternalInput").ap()
    b1c = nc.dram_tensor("b1c", [H, 1], f32, kind="ExternalInput").ap()
    b2c = nc.dram_tensor("b2c", [OUT, 1], f32, kind="ExternalInput").ap()
    # output spread over four 32-aligned partition bands (tile t -> band
    # 32*(t%4) rows 0..9, column slot t//4): a [10, B_SH] layout maps to ~3
    # SDMA engines and stores at ~40GB/s; full-width 128-partition stores
    # drain in parallel (the 22 pad rows per band are dead bytes, but HBM
    # writes are idle at that point and bandwidth beats byte count).
    # Stored as fp16 (host upcasts): output values are O(10), so fp16 adds
    # ~3e-4 relative error against the 1.3e-2 budget and halves store bytes.
    out = nc.dram_tensor("out", [KT, 4 * NB], f16,
                         kind="ExternalOutput").ap()

    with tile.TileContext(nc) as tc:
        with (
            tc.tile_pool(name="wpool", bufs=1) as wpool,
            tc.tile_pool(name="hpool", bufs=4) as hpool,
            tc.tile_pool(name="ps1", bufs=5, space="PSUM") as ps1pool,
            tc.tile_pool(name="ps2", bufs=3, space="PSUM") as ps2pool,
        ):
            x_sb = wpool.tile([KT, NT, NK, NB], fx)
            w_sb = wpool.tile([KT, NK + 1, H], f16)
            b1_sb = wpool.tile([H, 1], f32)
            b2_sb = wpool.tile([OUT, 1], f32)
            o_big = wpool.tile([KT, 4 * NB], f16)

            # sync ring: tile 0 in two k-chunks (the first matmul starts on
            # chunk 1 ~2us earlier than on a full tile; more chunks lose —
            # each DMA pays ~2us of completion-receipt latency), then solo
            # tiles 2/4 (the early phase is arrival-bound), pairs once
            # arrival outpaces the PE, then the output stores.  Ring FIFO
            # order = program order.
            for ka, kb in ((0, 3), (3, 7)):
                nc.sync.dma_start(x_sb[:, 0:1, ka:kb, :], xtp[:, 0:1, ka:kb, :])
            for a, b_ in ((2, 3), (4, 5), (6, 8), (10, 12), (14, 16)):
                nc.sync.dma_start(x_sb[:, a:b_], xtp[:, a:b_])
            # scalar ring: all weights in one contiguous DMA, tile 1's head
            # chunk, solo tiles 3/5, then pairs
            nc.scalar.dma_start(w_sb[:], wtp[:])
            nc.scalar.dma_start(x_sb[:, 1:2, 0:4, :], xtp[:, 1:2, 0:4, :])
            for a, b_ in ((3, 4), (5, 6), (8, 10), (12, 14)):
                nc.scalar.dma_start(x_sb[:, a:b_], xtp[:, a:b_])
            # tile 1's tail chunk rides SWDGE — the early phase is
            # aperture-bound on the two HWDGE rings, and 192KB fits the
            # slower gpsimd path's deadline (~13.9us).  Tiny biases follow;
            # their RMW-slow descriptors never sit in front of an x tile.
            nc.gpsimd.dma_start(x_sb[:, 1:2, 4:7, :], xtp[:, 1:2, 4:7, :])
            nc.gpsimd.dma_start(b1_sb[:], b1c[:])
            nc.gpsimd.dma_start(b2_sb[:], b2c[:])

            # PE pre-warm: dummy matmuls on a zeroed tile trip the HAM
            # activity monitor toward full clock before real data arrives.
            warm_x = wpool.tile([KT, NB], f16)
            nc.vector.memset(warm_x[:], 0.0)
            warm_ps = ps1pool.tile([H, NB], f32, tag="ps1")
            for _ in range(WARMUP):
                nc.tensor.matmul(warm_ps[:, 0:WARM_N], lhsT=warm_x[:, 0:H],
                                 rhs=warm_x[:, 0:WARM_N], start=True, stop=True)

            def epilogue(t, ps1, widths=(NB,)):
                # h = relu(ps1 + b1), fused on DVE, emitted as fp16; the
                # last tiles run in chunks so their serial h->mm2->o chain
                # pipelines instead of paying three full-width ops
                g, slot = t % 4, t // 4
                edges = [sum(widths[:i]) for i in range(len(widths) + 1)]
                for s in range(len(widths)):
                    cs = slice(edges[s], edges[s + 1])
                    h_sb = hpool.tile([H, NB], f16)
                    nc.vector.tensor_scalar(
                        h_sb[:, cs], ps1[:, cs], b1_sb[:], 0.0,
                        mybir.AluOpType.add, mybir.AluOpType.max)
                    # out.T[0:10, :] = w2 @ h.T  (M padded to 128)
                    ps2 = ps2pool.tile([128, NB], f32)
                    nc.tensor.matmul(ps2[:, cs], lhsT=w_sb[:, NK, :],
                                     rhs=h_sb[:, cs], start=True, stop=True)
                    # +b2 on the (otherwise idle) ScalarE so the DVE queue
                    # never delays the next tile's h
                    nc.scalar.activation(
                        o_big[32 * g:32 * g + OUT,
                              slot * NB + edges[s]:slot * NB + edges[s + 1]],
                        ps2[0:OUT, cs],
                        mybir.ActivationFunctionType.Identity, bias=b2_sb[:])

            pending = None  # software pipeline: tile t's epilogue is emitted
                            # after tile t+1's mm1 block so PE never waits on
                            # the DVE relu chain
            for t in range(NT):
                # h.T[128, NB] = W_eff.T @ x.T, accumulated over 7 K-tiles.
                ps1 = ps1pool.tile([H, NB], f32)
                for k in range(NK):
                    nc.tensor.matmul(
                        ps1[:],
                        lhsT=w_sb[:, k, :],
                        rhs=x_sb[:, t, k, :],
                        start=(k == 0),
                        stop=(k == NK - 1),
                    )
                if pending is not None:
                    epilogue(*pending,
                             widths=(NB // 2, NB // 2)
                             if pending[0] >= NT - 2 else (NB,))
                pending = (t, ps1)
            # asymmetric final split: the tail chain after the last matmul
            # only pays for a 128-column h->mm2->o before the last store
            epilogue(*pending, widths=(3 * NB // 4, NB // 4))

            # stores on the sync ring (HWDGE, idle after its x tiles):
            # tiles 0..7, then 8..11, fire mid-loop; slot 3 (tiles 12..15)
            # goes in column halves so the final store only needs tile 15's
            # first epilogue split plus a 128KB drain
            nc.sync.dma_start(out[:, 0:2 * NB], o_big[:, 0:2 * NB])
            nc.sync.dma_start(out[:, 2 * NB:3 * NB], o_big[:, 2 * NB:3 * NB])
            hb = 3 * NB + NB // 2
            nc.sync.dma_start(out[:, 3 * NB:hb], o_big[:, 3 * NB:hb])
            nc.sync.dma_start(out[:, hb:4 * NB], o_big[:, hb:4 * NB])

    nc.compile()
    return nc


def _get_nc():
    if "nc" not in _CACHE:
        _CACHE["nc"] = _build_nc()
    return _CACHE["nc"]


def _fold_weights(conv_w: np.ndarray, w1: np.ndarray) -> np.ndarray:
    """W_eff[784, 128]: h_pre = x @ W_eff  ==  conv(x) @ w1.T  (float64 accum)."""
    w1k = w1.reshape(H, 26, 26).transpose(1, 2, 0).astype(np.float64)  # [i,j,k]
    cw = conv_w.astype(np.float64)
    W = np.zeros((28, 28, H), np.float64)
    for di in range(3):
        for dj in range(3):
            W[di:di + 26, dj:dj + 26, :] += cw[di, dj] * w1k
    return W.reshape(D, H).astype(np.float32)


def make_in_maps(x, conv_w, w1, b1, w2, b2):
    x = np.asarray(x, np.float32)
    xdt = ml_dtypes.float8_e3m4 if X_DT == "f8" else np.float16
    weff = _fold_weights(np.asarray(conv_w, np.float32),
                         np.asarray(w1, np.float32))  # [784, 128] f32
    wtp = np.zeros((KT, NK + 1, H), np.float16)
    wtp[:, :6, :] = weff[:768].reshape(6, KT, H).transpose(1, 0, 2)
    wtp[0:16, 6, :] = weff[768:]
    wtp[:, 7, :OUT] = np.asarray(w2, np.float32).T  # [h, out] block
    b1c = np.ascontiguousarray(np.asarray(b1, np.float32).reshape(H, 1))
    b2c = np.ascontiguousarray(np.asarray(b2, np.float32).reshape(OUT, 1))
    wtp = np.ascontiguousarray(wtp)
    in_maps = []
    for i in range(N_CORES):
        xs = x[i * B_SH:(i + 1) * B_SH].astype(xdt)  # [8192, 784]
        xtp = np.zeros((KT, NT, NK, NB), xdt)
        # main: [t*NB+c, k*KT+p] -> [p, t, k, c]
        xtp[:, :, :6, :] = xs[:, :768].reshape(NT, NB, 6, KT).transpose(3, 0, 2, 1)
        xtp[0:16, :, 6, :] = xs[:, 768:].reshape(NT, NB, 16).transpose(2, 0, 1)
        in_maps.append({"xtp": np.ascontiguousarray(xtp),
                        "wtp": wtp, "b1c": b1c, "b2c": b2c})
    return in_maps


def kernel(x, conv_w, w1, b1, w2, b2):
    nc = _get_nc()
    in_maps = make_in_maps(x, conv_w, w1, b1, w2, b2)
    res = run_bass_kernel_spmd(nc, in_maps, list(range(N_CORES)))
    outs = []
    for i in range(N_CORES):
        o = res.results[i]["out"].astype(np.float32)
        # [128, 2048]: [32*(t%4)+j, (t//4)*512+b]
        v = o.reshape(4, 32, 4, NB)[:, :OUT]          # [g, j, slot, b]
        outs.append(v.transpose(2, 0, 3, 1).reshape(B_SH, OUT))
    return np.ascontiguousarray(np.concatenate(outs, axis=0))  # [65536, 10]
